# revision 1
# baseline (speedup 1.0000x reference)
"""Multi-head self-attention (B=4, S=2048, D=1024, H=16) on 8 TRN2 NeuronCores.

Sharding: batch x head-group. Core c handles batch b=c//2 and heads
[8*(c%2), 8*(c%2)+8). Each core computes QKV projection, attention and its
partial output projection; the host sums the two head-group partials per batch
and adds b_proj.

Per-core dataflow (all matmuls float32r = full PE rate, ~1.5e-4 rounding):
  stage 1: Y^T = [Q^T; K^T] = wqk^T-free matmul(lhsT=wqk, rhs=x^T) -> [1024f, 2048t]
           V   = matmul(lhsT=x^T chunk, rhs=wv)                    -> [2048t, 512f]
  stage 2: S^T[k,q] = K_h Q_h^T via row-tiled head pairs (d=64 contraction)
           P^T = exp(S^T * 0.125) on ACT (ScalarE), reading 2-bank PSUM tiles
  stage 3: C~^T = [V_h|1]^T P^T  (ones-column makes row 64 the softmax denom)
           normalize: recip(sums) -> DRAM -> partition-broadcast DMA -> DVE mul
  stage 4: out = C^T-proj: matmul(lhsT=C^T chunk, rhs=w_proj rows)  -> [2048t, 1024]
"""
import numpy as np

import concourse.bacc as bacc
import concourse.tile as tile
from concourse import bass_isa, mybir
from concourse import bass_utils

P = 128
B, S, D = 4, 2048, 1024
H_TOT, HD = 16, 64
H = 8          # heads per core
NPAIR = 4      # head pairs per core
SCALE = HD ** -0.5
DCH = D // P   # 8 contraction chunks
NTT = S // P   # 16 token tiles
f32 = mybir.dt.float32
f32r = mybir.dt.float32r
AF = mybir.ActivationFunctionType

_CACHED_NC = None


def build_nc():
    nc = bacc.Bacc(trn_type="TRN2", target_bir_lowering=False, debug=False)
    xt = nc.dram_tensor("xt", [D, S], f32r, kind="ExternalInput").ap()
    wqk = nc.dram_tensor("wqk", [D, 2 * H * HD], f32r, kind="ExternalInput").ap()
    wv = nc.dram_tensor("wv", [D, H * HD], f32r, kind="ExternalInput").ap()
    wp = nc.dram_tensor("wp", [H * HD, D], f32r, kind="ExternalInput").ap()
    bqk = nc.dram_tensor("bqk", [8, P], f32, kind="ExternalInput").ap()
    vbias = nc.dram_tensor("vbias", [P, NPAIR * 130], f32, kind="ExternalInput").ap()
    out = nc.dram_tensor("out", [S, D], f32, kind="ExternalOutput").ap()

    with tile.TileContext(nc) as tc:
        with tc.tile_pool(name="persist", bufs=1) as pp:
            # persistent SBUF tensors
            yt = [pp.tile([P, S], f32r, name=f"yt{f}") for f in range(8)]
            vp = pp.tile([P, NTT, NPAIR * 130], f32r, name="vp")
            vbias_t = pp.tile([P, NPAIR * 130], f32, name="vbias_t")

            # ---------------- stage 1: QKV projections ----------------
            with (
                tc.tile_pool(name="s1w", bufs=1) as s1w,
                tc.tile_pool(name="s1x", bufs=2) as s1x,
                tc.tile_pool(name="s1ps", bufs=4, space="PSUM") as s1ps,
            ):
                TCH = 256
                NCH = S // TCH
                # per-feature-tile weight tiles so the first matmuls start early
                wqk_f = [s1w.tile([P, DCH, P], f32r, name=f"wqkf{f}")
                         for f in range(8)]
                wv_t = s1w.tile([P, DCH, H * HD], f32r, name="wv_t")
                bqk_t = s1w.tile([P, 8], f32, name="bqk_t")
                wqk4 = wqk.rearrange("(c p) (f g) -> p c f g", p=P, f=8)
                # first xt chunks + weight tiles up front; weights go on the
                # ACT HWDGE ring so they don't queue behind the xt stream
                xt_ts = [s1x.tile([P, DCH, TCH], f32r, name="xt_t")
                         for _ in range(2)]
                nc.sync.dma_start(xt_ts[0][:],
                                  xt[:, 0:TCH].rearrange("(c p) s -> p c s", p=P))
                nc.scalar.dma_start(wqk_f[0][:], wqk4[:, :, 0, :])
                nc.scalar.dma_start(bqk_t[:], bqk.rearrange("a p -> p a"))
                nc.scalar.dma_start(vbias_t[:], vbias[:])
                for f in range(1, 8):
                    nc.scalar.dma_start(wqk_f[f][:], wqk4[:, :, f, :])
                nc.sync.dma_start(
                    xt_ts[1][:],
                    xt[:, TCH:2 * TCH].rearrange("(c p) s -> p c s", p=P))
                nc.scalar.dma_start(wv_t[:], wv.rearrange("(c p) f -> p c f", p=P))

                for t in range(NCH):  # 256-token chunks
                    tsl = slice(t * TCH, (t + 1) * TCH)
                    if t < 2:
                        xt_t = xt_ts[t]
                    else:
                        xt_t = s1x.tile([P, DCH, TCH], f32r, name="xt_t")
                        nc.sync.dma_start(
                            xt_t[:], xt[:, tsl].rearrange("(c p) s -> p c s", p=P))
                    for f in range(8):  # Q,K feature tiles
                        ps = s1ps.tile([P, TCH], f32, name="s1pq")
                        for i in range(DCH):
                            nc.tensor.matmul(
                                ps[:], wqk_f[f][:, i, :], xt_t[:, i, :],
                                start=(i == 0), stop=(i == DCH - 1))
                        nc.vector.tensor_scalar(
                            out=yt[f][:, tsl], in0=ps[:], scalar1=bqk_t[:, f:f + 1],
                            scalar2=None, op0=mybir.AluOpType.add)
                    if t == 0:
                        # vp bias+ones init, deferred so the xt/weight streams
                        # get the startup DMA bandwidth
                        for tt2 in range(NTT):
                            nc.gpsimd.dma_start(vp[:, tt2, :], vbias[:])
                    for sub in range(TCH // P):  # V for 128-token subtiles
                        tt = t * (TCH // P) + sub
                        ps = s1ps.tile([P, 512], f32, name="s1p")
                        for i in range(DCH):
                            nc.tensor.matmul(
                                ps[:], xt_t[:, i, sub * P:(sub + 1) * P], wv_t[:, i, :],
                                start=(i == 0), stop=(i == DCH - 1))
                        vpt = vp[:, tt, :].rearrange("p (j k c) -> p j k c",
                                                     j=NPAIR, k=2)
                        vb4 = vbias_t[:].rearrange("p (j k c) -> p j k c",
                                                   j=NPAIR, k=2)
                        nc.vector.tensor_tensor(
                            out=vpt[:, :, :, 0:HD],
                            in0=ps[:].rearrange("p (j k c) -> p j k c", j=NPAIR, k=2),
                            in1=vb4[:, :, :, 0:HD],
                            op=mybir.AluOpType.add)

            # ---------------- stages 2+3: attention ----------------
            # Per k-chunk: one [128,1024] PSUM tile holds S^T for both heads of
            # the pair (even in cols 0:512, odd in 512:1024), one ACT exp per
            # k-chunk, PV software-pipelined one k-chunk behind.
            ct = [pp.tile([P, S], f32r, name=f"ct{j}") for j in range(NPAIR)]
            with (
                tc.tile_pool(name="s4w", bufs=1) as s4w,
                tc.tile_pool(name="att", bufs=1) as att,
                tc.tile_pool(name="s4o", bufs=2) as s4o,
                tc.tile_pool(name="spt", bufs=2, space="PSUM") as sptp,
                tc.tile_pool(name="cps", bufs=2, space="PSUM") as cpsp,
                tc.tile_pool(name="s4ps", bufs=2, space="PSUM") as s4ps,
            ):
                wp_t = s4w.tile([P, NPAIR, D], f32r, name="wp_t")
                nc.scalar.dma_start(wp_t[:], wp.rearrange("(c p) f -> p c f", p=P))

                # zeros rows 0:63 + per-norm recip row 64; partition all-reduce
                # (add) then replicates the recip row across all partitions
                zt = att.tile([65, 1024], f32, name="zt", bufs=1)
                nc.vector.memset(zt[0:HD, :], 0.0)

                def emit_norm(j, qa, cps_e, cps_o):
                    nc.vector.reciprocal(zt[64:65, 0:512], cps_e[64:65, :])
                    nc.vector.reciprocal(zt[64:65, 512:1024], cps_o[64:65, :])
                    rbc = att.tile([65, 1024], f32, name="rbc", bufs=2)
                    nc.gpsimd.partition_all_reduce(
                        rbc[:], zt[:], channels=65,
                        reduce_op=bass_isa.ReduceOp.add)
                    nc.vector.tensor_mul(ct[j][0:HD, qa], cps_e[0:HD, :],
                                         rbc[0:HD, 0:512])
                    cttmp = att.tile([HD, 512], f32r, name="cttmp", bufs=1)
                    nc.vector.tensor_mul(cttmp[:], cps_o[0:HD, :],
                                         rbc[0:HD, 512:1024])
                    nc.sync.dma_start(ct[j][HD:P, qa], cttmp[:])

                # projection work for one token tile, emitted as a list of
                # closures so matmuls drip into the PE stream without bursts
                def proj_steps(tt):
                    tsl = slice(tt * P, (tt + 1) * P)
                    steps = []
                    state = {}

                    def mk_mm(half, fc):
                        def f():
                            if fc == 0:
                                state[half] = s4ps.tile([P, 512], f32, name="s4p")
                            nc.tensor.matmul(
                                state[half][:], ct[fc][:, tsl],
                                wp_t[:, fc, half * 512:(half + 1) * 512],
                                start=(fc == 0), stop=(fc == NPAIR - 1))
                            if fc == NPAIR - 1:
                                o_sb = s4o.tile([P, 512], f32, name="o_sb",
                                                bufs=4)
                                nc.vector.tensor_copy(o_sb[:], state[half][:])
                                nc.sync.dma_start(
                                    out[tsl, half * 512:(half + 1) * 512],
                                    o_sb[:])
                        return f

                    for half in range(2):
                        for fc in range(NPAIR):
                            steps.append(mk_mm(half, fc))
                    return steps

                norm_pending = None
                proj_queue = []
                for qc in range(4):  # 512-wide query chunks, outer
                    qa = slice(qc * 512, (qc + 1) * 512)
                    for j in range(NPAIR):
                        qt, kt = yt[j], yt[NPAIR + j]
                        cps_e = cps_o = None
                        pv_pending = None
                        for kc in range(NTT):
                            ksl = slice(kc * P, (kc + 1) * P)
                            spt = sptp.tile([P, 1024], f32, name="spt")
                            nc.tensor.matmul(spt[:, 0:512], kt[0:HD, ksl],
                                             qt[0:HD, qa], start=True, stop=True)
                            nc.tensor.matmul(spt[:, 512:1024], kt[HD:P, ksl],
                                             qt[HD:P, qa], start=True, stop=True)
                            ppt = att.tile([P, 1024], f32r, name="ppt", bufs=4)
                            nc.scalar.activation(ppt[:], spt[:], AF.Exp,
                                                 scale=SCALE)
                            if kc == 1 and norm_pending is not None:
                                # previous (qc,j) normalization, deferred past
                                # this iteration's first two S/exp to hide its
                                # recip -> all-reduce -> mul chain
                                emit_norm(*norm_pending)
                                norm_pending = None
                            if pv_pending is not None:
                                if cps_e is None:
                                    cps_e = cpsp.tile([65, 512], f32, name="cps")
                                    cps_o = cpsp.tile([65, 512], f32, name="cps")
                                _emit_pv(nc, cps_e, cps_o, vp, pv_pending[0],
                                         pv_pending[1], j)
                            pv_pending = (kc, ppt)
                            if proj_queue and kc % 2 == 1:
                                proj_queue.pop(0)()  # drip one projection step
                        _emit_pv(nc, cps_e, cps_o, vp, pv_pending[0],
                                 pv_pending[1], j)
                        norm_pending = (j, qa, cps_e, cps_o)
                    # queue projection for this query chunk's 4 token tiles
                    # (runnable once this qc's last norm flushes next sweep)
                    for tt in range(qc * 4, (qc + 1) * 4):
                        proj_queue.extend(proj_steps(tt))
                emit_norm(*norm_pending)
                for step in proj_queue:
                    step()

    nc.finalize()
    return nc


def _emit_pv(nc, cps_e, cps_o, vp, kc, ppt, j):
    nc.tensor.matmul(cps_e[0:65, :], vp[:, kc, j * 130:j * 130 + 65],
                     ppt[:, 0:512], start=(kc == 0), stop=(kc == NTT - 1))
    nc.tensor.matmul(cps_o[0:65, :], vp[:, kc, j * 130 + 65:j * 130 + 130],
                     ppt[:, 512:1024], start=(kc == 0), stop=(kc == NTT - 1))


def get_nc():
    global _CACHED_NC
    if _CACHED_NC is None:
        _CACHED_NC = build_nc()
    return _CACHED_NC


def make_in_maps(x, w_qkv, b_qkv, w_proj):
    """Host-side sharding: one input dict per core."""
    w = np.asarray(w_qkv, np.float32).reshape(D, 3, H_TOT, HD)
    bq3 = np.asarray(b_qkv, np.float32).reshape(3, H_TOT, HD)
    in_maps = []
    for c in range(8):
        b, hg = c // 2, c % 2
        hs = slice(hg * H, (hg + 1) * H)
        wqk_c = np.ascontiguousarray(
            np.concatenate([w[:, 0, hs, :].reshape(D, H * HD),
                            w[:, 1, hs, :].reshape(D, H * HD)], axis=1))
        wv_c = np.ascontiguousarray(w[:, 2, hs, :].reshape(D, H * HD))
        wp_c = np.ascontiguousarray(
            np.asarray(w_proj, np.float32).reshape(H_TOT, HD, D)[hs].reshape(H * HD, D))
        bqk_c = np.ascontiguousarray(
            np.concatenate([bq3[0, hs].reshape(H * HD),
                            bq3[1, hs].reshape(H * HD)]).reshape(8, P))
        bv = bq3[2, hs].reshape(H * HD)
        vbias_c = np.zeros((P, NPAIR * 130), np.float32)
        for j in range(NPAIR):
            vbias_c[:, j * 130:j * 130 + HD] = bv[(2 * j) * HD:(2 * j + 1) * HD]
            vbias_c[:, j * 130 + HD] = 1.0
            vbias_c[:, j * 130 + 65:j * 130 + 65 + HD] = \
                bv[(2 * j + 1) * HD:(2 * j + 2) * HD]
            vbias_c[:, j * 130 + 129] = 1.0
        xt_c = np.ascontiguousarray(np.asarray(x[b], np.float32).T)
        in_maps.append({"xt": xt_c, "wqk": wqk_c, "wv": wv_c, "wp": wp_c,
                        "bqk": bqk_c, "vbias": vbias_c})
    return in_maps


def assemble(results, b_proj):
    out = np.empty((B, S, D), np.float32)
    bp = np.asarray(b_proj, np.float32)
    for b in range(B):
        out[b] = results[2 * b]["out"] + results[2 * b + 1]["out"] + bp
    return out


def kernel(x, w_qkv, b_qkv, w_proj, b_proj):
    nc = get_nc()
    in_maps = make_in_maps(x, w_qkv, b_qkv, w_proj)
    res = bass_utils.run_bass_kernel_spmd(nc, in_maps, core_ids=list(range(8)),
                                          trace=False)
    return assemble(res.results, b_proj)



# revision 4
# speedup vs baseline: 5.3657x; 5.3657x over previous
"""Multi-head self-attention (B=4, S=2048, D=1024, H=16) on 8 TRN2 NeuronCores.

Sharding: head-pair tensor parallel + token-sharded I/O. Core c owns heads
{2c, 2c+1} for ALL batches; weights shard 8-way with zero duplication. The
input x is shipped 1/8 per core (1024 tokens, transposed, fp16) and
AllGathered on-chip; each core computes QKV, attention and its 2-head partial
projection for all 4 batches; a per-batch ReduceScatter sums the partials and
leaves each core with 256 final rows per batch (fp16). Host adds b_proj.

All host<->device I/O is fp16 (~5 MB/core vs 24.3 MB for the v0 kernel) —
per-run staging of kernel I/O dominates measured time at ~14 GB/s aggregate.

Per-core dataflow per batch b (matmuls fp16 operands, fp32 PSUM):
  stage 1: yt_q/yt_k = [Q^T;K^T] [128f, 2048t], vp = V+[bias|ones] [2048t,130]
           (emitted in 4 chunks, dripped between attention query sweeps)
  stage 2: S^T[k,q] for the head pair (d=64 contraction), exp on ACT -> fp16
  stage 3: C~^T = [V_h|1]^T P^T (psum row 64 = softmax denom);
           recip -> gpsimd partition-broadcast -> DVE normalize -> ct fp16
  stage 4: out_partial = ct^T @ wp -> fp16 -> DRAM, dripped between S/exp
           steps; ReduceScatter(batch) once its last tile lands
"""
import numpy as np

import concourse.bacc as bacc
import concourse.tile as tile
from concourse import bass_isa, mybir
from concourse import bass_utils

P = 128
B, S, D = 4, 2048, 1024
H_TOT, HD = 16, 64
SCALE = HD ** -0.5
SH_T = 1024        # tokens per shard (B*S/8)
DCH = D // P       # 8 contraction chunks
NTT = S // P       # 16 token tiles per batch
f32 = mybir.dt.float32
f16 = mybir.dt.float16
AF = mybir.ActivationFunctionType
RG8 = [[0, 1, 2, 3, 4, 5, 6, 7]]

_CACHED_NC = None


def build_nc():
    nc = bacc.Bacc(trn_type="TRN2", target_bir_lowering=False, debug=False,
                   num_devices=8)
    xs = nc.dram_tensor("xs", [D, SH_T], f16, kind="ExternalInput").ap()
    wqk = nc.dram_tensor("wqk", [D, 2 * P], f16, kind="ExternalInput").ap()
    wv = nc.dram_tensor("wv", [D, P], f16, kind="ExternalInput").ap()
    wp = nc.dram_tensor("wp", [P, D], f16, kind="ExternalInput").ap()
    bqk = nc.dram_tensor("bqk", [P, 2], f32, kind="ExternalInput").ap()
    vb = nc.dram_tensor("vb", [P, 130], f32, kind="ExternalInput").ap()
    out = nc.dram_tensor("out", [B * 256, D], f16, kind="ExternalOutput").ap()

    ag_in = nc.dram_tensor("ag_in", [D, SH_T], f16, kind="Internal").ap()
    ag_out = nc.dram_tensor("ag_out", [8 * D, SH_T], f16, kind="Internal",
                            addr_space="Shared").ap()
    rs_in = [nc.dram_tensor(f"rs_in{b}", [S, D], f16, kind="Internal").ap()
             for b in range(B)]
    rs_out = [nc.dram_tensor(f"rs_out{b}", [256, D], f16,
                             kind="Internal").ap() for b in range(B)]

    with tile.TileContext(nc) as tc:
        with tc.tile_pool(name="persist", bufs=1) as pp:
            # double-buffered per-batch persistent tensors (b%2)
            ytq = [pp.tile([P, S], f16, name=f"ytq{i}") for i in range(2)]
            ytk = [pp.tile([P, S], f16, name=f"ytk{i}") for i in range(2)]
            vp = [pp.tile([P, NTT, 130], f16, name=f"vp{i}") for i in range(2)]
            ct = [pp.tile([P, S], f16, name=f"ct{i}") for i in range(2)]
            wqk_t = pp.tile([P, DCH, 2 * P], f16, name="wqk_t")
            wv_t = pp.tile([P, DCH, P], f16, name="wv_t")
            wp_t = pp.tile([P, D], f16, name="wp_t")
            bqk_t = pp.tile([P, 2], f32, name="bqk_t")
            vb_t = pp.tile([P, 130], f32, name="vb_t")

            # stage 0: weights to SBUF; x shard -> internal dram -> AllGather
            nc.scalar.dma_start(wqk_t[:],
                                wqk.rearrange("(c p) f -> p c f", p=P))
            nc.scalar.dma_start(wv_t[:], wv.rearrange("(c p) f -> p c f", p=P))
            nc.scalar.dma_start(wp_t[:], wp[:])
            nc.scalar.dma_start(bqk_t[:], bqk[:])
            nc.scalar.dma_start(vb_t[:], vb[:])
            with tc.tile_pool(name="agb", bufs=1) as agb:
                agt = agb.tile([P, DCH, SH_T], f16, name="agt")
                nc.sync.dma_start(agt[:],
                                  xs.rearrange("(c p) t -> p c t", p=P))
                nc.sync.dma_start(ag_in.rearrange("(c p) t -> p c t", p=P),
                                  agt[:])
            nc.gpsimd.collective_compute(
                "AllGather", mybir.AluOpType.bypass, replica_groups=RG8,
                ins=[ag_in[:]], outs=[ag_out[:]])
            # the softmax-denominator ones columns of vp (written once)
            for i in range(2):
                nc.vector.memset(vp[i][:, :, HD:HD + 1], 1.0)
                nc.vector.memset(vp[i][:, :, 129:130], 1.0)

            with (
                tc.tile_pool(name="s1x", bufs=2) as s1x,
                tc.tile_pool(name="s1ps", bufs=1, space="PSUM") as s1ps,
                tc.tile_pool(name="att", bufs=1) as att,
                tc.tile_pool(name="s4o", bufs=2) as s4o,
                tc.tile_pool(name="spt", bufs=2, space="PSUM") as sptp,
                tc.tile_pool(name="cps", bufs=2, space="PSUM") as cpsp,
                tc.tile_pool(name="s4ps", bufs=1, space="PSUM") as s4ps,
            ):
                # zeros rows 0:64 + recip row 64; gpsimd partition all-reduce
                # broadcasts the recip row to all partitions
                zt = att.tile([65, 1024], f32, name="zt", bufs=1)
                nc.vector.memset(zt[0:HD, :], 0.0)

                def stage1_unit(b, half, tc_i):
                    """QKV projection for one 512-token chunk of batch b."""
                    yq, yk, vpb = ytq[b % 2], ytk[b % 2], vp[b % 2]
                    sh = 2 * b + half
                    rows = ag_out[sh * D:(sh + 1) * D, :]
                    tsl_l = slice(tc_i * 512, (tc_i + 1) * 512)
                    tsl_g = slice(half * SH_T + tc_i * 512,
                                  half * SH_T + (tc_i + 1) * 512)
                    xt_t = s1x.tile([P, DCH, 512], f16, name="xt_t")
                    nc.sync.dma_start(
                        xt_t[:],
                        rows[:, tsl_l].rearrange("(c p) t -> p c t", p=P))
                    for qk in range(2):  # Q then K features
                        ps = s1ps.tile([P, 512], f32, name="s1p")
                        for i in range(DCH):
                            nc.tensor.matmul(
                                ps[:], wqk_t[:, i, qk * P:(qk + 1) * P],
                                xt_t[:, i, :],
                                start=(i == 0), stop=(i == DCH - 1))
                        ydst = (yq if qk == 0 else yk)
                        nc.vector.tensor_scalar(
                            out=ydst[:, tsl_g], in0=ps[:],
                            scalar1=bqk_t[:, qk:qk + 1],
                            scalar2=None, op0=mybir.AluOpType.add)
                    for sub in range(4):  # V for 128-token subtiles
                        tt = (2 * half + tc_i) * 4 + sub
                        ps = s1ps.tile([P, 512], f32, name="s1p")
                        for i in range(DCH):
                            nc.tensor.matmul(
                                ps[:, 0:P], xt_t[:, i, sub * P:(sub + 1) * P],
                                wv_t[:, i, :],
                                start=(i == 0), stop=(i == DCH - 1))
                        vpt = vpb[:, tt, :].rearrange("p (k c) -> p k c", k=2)
                        vb4 = vb_t[:].rearrange("p (k c) -> p k c", k=2)
                        nc.vector.tensor_tensor(
                            out=vpt[:, :, 0:HD],
                            in0=ps[:, 0:P].rearrange("p (k c) -> p k c", k=2),
                            in1=vb4[:, :, 0:HD],
                            op=mybir.AluOpType.add)

                def emit_norm(b, qa, cps_e, cps_o):
                    ctb = ct[b % 2]
                    nc.vector.reciprocal(zt[64:65, 0:512], cps_e[64:65, :])
                    nc.vector.reciprocal(zt[64:65, 512:1024], cps_o[64:65, :])
                    rbc = att.tile([65, 1024], f32, name="rbc", bufs=2)
                    nc.gpsimd.partition_all_reduce(
                        rbc[:], zt[:], channels=65,
                        reduce_op=bass_isa.ReduceOp.add)
                    nc.vector.tensor_mul(ctb[0:HD, qa], cps_e[0:HD, :],
                                         rbc[0:HD, 0:512])
                    cttmp = att.tile([HD, 512], f16, name="cttmp", bufs=1)
                    nc.vector.tensor_mul(cttmp[:], cps_o[0:HD, :],
                                         rbc[0:HD, 512:1024])
                    nc.sync.dma_start(ctb[HD:P, qa], cttmp[:])

                # one projection token tile half -> rs_in rows
                def proj_step(b, tt, half):
                    def f():
                        tsl = slice(tt * P, (tt + 1) * P)
                        ps = s4ps.tile([P, 512], f32, name="s4p")
                        nc.tensor.matmul(
                            ps[:], ct[b % 2][:, tsl],
                            wp_t[:, half * 512:(half + 1) * 512],
                            start=True, stop=True)
                        o_sb = s4o.tile([P, 512], f16, name="o_sb", bufs=4)
                        nc.vector.tensor_copy(o_sb[:], ps[:])
                        nc.sync.dma_start(
                            rs_in[b][tt * P:(tt + 1) * P,
                                     half * 512:(half + 1) * 512],
                            o_sb[:])
                    return f

                def emit_pv(cps_e, cps_o, vpb, kc, ppt):
                    nc.tensor.matmul(cps_e[:], vpb[:, kc, 0:65],
                                     ppt[:, 0:512],
                                     start=(kc == 0), stop=(kc == NTT - 1))
                    nc.tensor.matmul(cps_o[:], vpb[:, kc, 65:130],
                                     ppt[:, 512:1024],
                                     start=(kc == 0), stop=(kc == NTT - 1))

                def emit_rs(b):
                    nc.gpsimd.collective_compute(
                        "ReduceScatter", mybir.AluOpType.add,
                        replica_groups=RG8,
                        ins=[rs_in[b][:]], outs=[rs_out[b][:]])
                    ob = s4o.tile([P, 2, D], f16, name="ob", bufs=2)
                    nc.sync.dma_start(
                        ob[:],
                        rs_out[b][:].rearrange("(k p) d -> p k d", p=P))
                    nc.sync.dma_start(
                        out[b * 256:(b + 1) * 256, :].rearrange(
                            "(k p) d -> p k d", p=P), ob[:])

                for u in range(4):
                    stage1_unit(0, u // 2, u % 2)
                norm_pending = None
                proj_queue = []   # (closure, rs_batch_or_None)
                rs_ready = []
                s1_queue = []
                for b in range(B):
                    if b + 1 < B:
                        s1_queue = [(b + 1, u // 2, u % 2) for u in range(4)]
                    yq, yk, vpb = ytq[b % 2], ytk[b % 2], vp[b % 2]
                    for qc in range(4):  # 512-wide query chunks
                        qa = slice(qc * 512, (qc + 1) * 512)
                        cps_e = cps_o = None
                        pv_pending = None
                        for kc in range(NTT):
                            ksl = slice(kc * P, (kc + 1) * P)
                            spt = sptp.tile([P, 1024], f32, name="spt")
                            nc.tensor.matmul(spt[:, 0:512], yk[0:HD, ksl],
                                             yq[0:HD, qa],
                                             start=True, stop=True)
                            nc.tensor.matmul(spt[:, 512:1024], yk[HD:P, ksl],
                                             yq[HD:P, qa],
                                             start=True, stop=True)
                            ppt = att.tile([P, 1024], f16, name="ppt", bufs=4)
                            nc.scalar.activation(ppt[:], spt[:], AF.Exp,
                                                 scale=SCALE)
                            if kc == 1 and norm_pending is not None:
                                emit_norm(*norm_pending)
                                norm_pending = None
                                if rs_ready:
                                    emit_rs(rs_ready.pop(0))
                            if pv_pending is not None:
                                if cps_e is None:
                                    cps_e = cpsp.tile([65, 512], f32,
                                                      name="cps")
                                    cps_o = cpsp.tile([65, 512], f32,
                                                      name="cps")
                                emit_pv(cps_e, cps_o, vpb, pv_pending[0],
                                        pv_pending[1])
                            pv_pending = (kc, ppt)
                            if proj_queue and kc >= 3:
                                fn, rsb = proj_queue.pop(0)
                                fn()
                                if rsb is not None:
                                    rs_ready.append(rsb)
                        emit_pv(cps_e, cps_o, vpb, pv_pending[0],
                                pv_pending[1])
                        norm_pending = (b, qa, cps_e, cps_o)
                        for tt in range(qc * 4, (qc + 1) * 4):
                            proj_queue.append((proj_step(b, tt, 0), None))
                            proj_queue.append((proj_step(b, tt, 1), None))
                        if s1_queue:  # drip next batch's QKV between sweeps
                            stage1_unit(*s1_queue.pop(0))
                    # tag batch b's last projection step so the RS fires
                    # once it has drained (at a later norm point)
                    fn0, _ = proj_queue[-1]
                    proj_queue[-1] = (fn0, b)
                emit_norm(*norm_pending)
                for fn, rsb in proj_queue:
                    fn()
                    if rsb is not None:
                        rs_ready.append(rsb)
                for rsb in rs_ready:
                    emit_rs(rsb)

    nc.finalize()
    return nc


def get_nc():
    global _CACHED_NC
    if _CACHED_NC is None:
        _CACHED_NC = build_nc()
    return _CACHED_NC


def make_in_maps(x, w_qkv, b_qkv, w_proj):
    """Host-side sharding: one input dict per core (all tensor I/O fp16)."""
    xf = np.asarray(x, np.float32).reshape(B * S, D)
    w3 = np.asarray(w_qkv, np.float32).reshape(D, 3, H_TOT, HD)
    b3 = np.asarray(b_qkv, np.float32).reshape(3, H_TOT, HD)
    wpr = np.asarray(w_proj, np.float32).reshape(H_TOT, HD, D)
    in_maps = []
    for c in range(8):
        hs = slice(2 * c, 2 * c + 2)
        xs_c = np.ascontiguousarray(
            xf[c * SH_T:(c + 1) * SH_T].T.astype(np.float16))
        wqk_c = np.ascontiguousarray(np.concatenate(
            [w3[:, 0, hs].reshape(D, P), w3[:, 1, hs].reshape(D, P)],
            axis=1).astype(np.float16))
        wv_c = np.ascontiguousarray(w3[:, 2, hs].reshape(D, P)
                                    .astype(np.float16))
        wp_c = np.ascontiguousarray(wpr[hs].reshape(P, D).astype(np.float16))
        bqk_c = np.ascontiguousarray(np.stack(
            [b3[0, hs].reshape(P), b3[1, hs].reshape(P)], axis=1)
            .astype(np.float32))
        vb_c = np.zeros((P, 130), np.float32)
        vb_c[:, 0:HD] = b3[2, 2 * c]
        vb_c[:, 65:65 + HD] = b3[2, 2 * c + 1]
        in_maps.append({"xs": xs_c, "wqk": wqk_c, "wv": wv_c, "wp": wp_c,
                        "bqk": bqk_c, "vb": vb_c})
    return in_maps


def assemble(results, b_proj):
    out = np.empty((B, S, D), np.float32)
    bp = np.asarray(b_proj, np.float32)
    for c in range(8):
        oc = np.asarray(results[c]["out"], np.float32)  # [B*256, D]
        for b in range(B):
            out[b, c * 256:(c + 1) * 256] = oc[b * 256:(b + 1) * 256]
    return out + bp


def kernel(x, w_qkv, b_qkv, w_proj, b_proj):
    nc = get_nc()
    in_maps = make_in_maps(x, w_qkv, b_qkv, w_proj)
    res = bass_utils.run_bass_kernel_spmd(nc, in_maps, core_ids=list(range(8)),
                                          trace=False)
    return assemble(res.results, b_proj)
